# revision 35
# baseline (speedup 1.0000x reference)
"""CapsNet DigitCaps routing kernel for 8 TRN2 NeuronCores — v2.

Strategy: shard the 1152 primary capsules across the 8 cores (144 each),
keep the full batch (256, as two 128-row halves bt) on every core.

v2 never materializes u_hat. Per routing iteration:

  s-phase:   s[b,d,i] = sum_{p,j} W[d,p,i,j] * (c[b,d,p] * x[b,p,j])
    xc = c (*) x is a single broadcast multiply in j-space (8 wide, half
    the elements of c*u_hat); xc is PE-transposed per digit into (j,p)-
    partition tiles, drained to SBUF, and contracted with a host-packed
    W operand on the TensorEngine (9 accumulating K=128, N=16 matmuls
    per (bt,d)) straight into s[b, (d,i)] layout.
    At r=0, c == 0.1 uniformly, so the transposed x (times 0.1) is a
    static input and the s-phase is matmuls only.

  squash:    global-Frobenius-norm alpha; the per-core partial s is
    AllReduced (per-bt round trips so bt0's collective overlaps bt1's
    compute), then every core computes the same
    alpha = n2 / ((n2+1)(sqrt(n2)+eps)).

  g-phase (j-space): wv[b,d,p,j] = sum_i W[d,p,i,j] * s[b,d,i] on the
    TensorEngine (lhsT = s^T tiles from a post-AllReduce PE transpose),
    output kept bf16 in PSUM; g_raw = sum_j x (*) wv via a DVE/Pool
    multiply that reads PSUM directly plus a j-tree. alpha is never
    applied to wv or g_raw: the routing state is kept as
    E = prod_r exp(alpha_r * g_raw_r), updated with exp(scale=alpha)
    on the Activation engine and one bf16 multiply — softmax c is then
    E * (1/sum_d E).

Layouts (per core, SBUF partition dim first, p = local primary index):
  xbp [128, 2*1152] bf16   col = bt*1152 + j*144 + p      (g multiply)
  xt9 [128, 9*256]  bf16   0.1*x^T: row = (j*144+p)%128, col = k*256+b
  wsc [128, 1440]   bf16   row = (j*144+p)%128, col = (d*9+k)*16 + i
  wg  [16, 11520]   bf16   row = i, col = d*1152 + j*144 + p
  s_sb [128, 2*160] f32    col = bt*160 + d*16 + i  (matches v_out!)
  E,G,c [128, *1440] bf16  col = d*144 + p (d-major)
  xc  [128, 11520]  bf16   col = d*1152 + j*144 + p (per bt)
  xcT [128, 11520]  bf16   col = (d*9+k)*128 + (jp%128), value xc^T
  vtd [16, 2560]    bf16   col = d*256 + bt*128 + b  (s^T for wv lhsT)
"""

import os
import sys

for _p in ("/opt/trn_rl_repo",):
    if _p not in sys.path and os.path.isdir(_p):
        sys.path.insert(0, _p)

import numpy as np
import ml_dtypes

import concourse.bass as bass
import concourse.bacc as bacc
import concourse.mybir as mybir
import concourse.tile as tile
from concourse.bass_utils import run_bass_kernel_spmd

F32 = mybir.dt.float32
BF16 = mybir.dt.bfloat16
MULT = mybir.AluOpType.mult
ADD = mybir.AluOpType.add
AF = mybir.ActivationFunctionType

B, D, P, I, J = 256, 10, 1152, 16, 8
CORES = 8
PL = P // CORES          # 144 local primary capsules
JP = J * PL              # 1152 (j,p) rows per digit
NK = JP // 128           # 9 partition chunks per digit
EPS = 1e-7
NROUT = 3

# routing rounds actually executed (debug bisection: 1, 2, or 3)
STAGE = int(os.environ.get("BASSCAPS_STAGE", "3"))
# Replace the AllReduce with a local DRAM copy (for TimelineSim profiling).
NO_CC = os.environ.get("BASSCAPS_NO_CC", "0") == "1"
# Pool engine shares
POOL_XC_D = int(os.environ.get("BASSCAPS_POOL_XC_D", "2"))   # digits of xc mult
POOL_G_D = int(os.environ.get("BASSCAPS_POOL_G_D", "2"))     # digits of g phase
DVE_DIRECT_D = int(os.environ.get("BASSCAPS_DVE_DIRECT_D", "1"))
# of the 12 xcT drain batches per bt: how many go to ACT / Pool (rest DVE)
DRAIN_ACT = int(os.environ.get("BASSCAPS_DRAIN_ACT", "6"))


def build_program():
    nc = bacc.Bacc("TRN2", target_bir_lowering=False, debug=False,
                   num_devices=CORES)

    xbp_d = nc.dram_tensor("x_bp", [128, 2 * JP], BF16, kind="ExternalInput")
    xt9_d = nc.dram_tensor("x_t9", [128, NK * 256], BF16, kind="ExternalInput")
    wsc_d = nc.dram_tensor("w_sc", [128, D * NK * 16], BF16,
                           kind="ExternalInput")
    wg_d = nc.dram_tensor("w_g", [128, D * JP], BF16, kind="ExternalInput")
    ident_d = nc.dram_tensor("ident", [128, 128], BF16, kind="ExternalInput")
    v_d = nc.dram_tensor("v_out", [B, D, I], F32, kind="ExternalOutput")

    NROUT_RUN = max(1, min(NROUT, STAGE))

    with tile.TileContext(nc) as tc:
        with (
            tc.tile_pool(name="persist", bufs=1) as pp,
            tc.tile_pool(name="psS", bufs=2, space=bass.MemorySpace.PSUM) as psS,
            tc.tile_pool(name="psW", bufs=5, space=bass.MemorySpace.PSUM) as psW,
            tc.tile_pool(name="psV", bufs=1, space=bass.MemorySpace.PSUM) as psV,
            tc.tile_pool(name="dram", bufs=1, space=bass.MemorySpace.DRAM) as dp,
        ):
            xbp = pp.tile([128, 2 * JP], BF16)
            xt9 = pp.tile([128, NK * 256], BF16)
            wsc = pp.tile([128, D * NK * 16], BF16)
            wg = pp.tile([128, D * JP], BF16)
            ident = pp.tile([128, 128], BF16)

            s_sb = pp.tile([128, 2 * 160], F32)
            sbb = pp.tile([128, 2 * 160], BF16)    # bf16 AllReduce payload
            sv = pp.tile([128, 2 * 160], F32)
            s_bf = pp.tile([128, 2 * 320], BF16)   # pad-32 staging for vt
            vtd = pp.tile([128, 4 * 256], BF16)
            E = pp.tile([128, 2 * 1440], BF16)
            Et = pp.tile([128, 2 * 1440], BF16)
            G = pp.tile([128, 2 * 1440], BF16)
            cb = pp.tile([128, 2 * 1440], BF16)
            zs = pp.tile([128, 2 * 720], BF16)
            zrec = pp.tile([128, 2 * 144], F32)
            zrecb = pp.tile([128, 2 * 144], BF16)
            xc = pp.tile([128, 2 * D * JP], BF16)   # per-bt halves
            xcT = pp.tile([128, 2 * D * JP], BF16)
            Y = pp.tile([128, 3 * JP], BF16)       # g-phase scratch (3 bufs)
            Yw = pp.tile([128, 3 * JP], BF16)      # wv drain staging

            sq = pp.tile([128, 2 * 160], F32)
            accb = pp.tile([128, 2], F32)
            acc = pp.tile([128, 1], F32)
            ones = pp.tile([128, 128], F32)
            n2sb = pp.tile([128, 1], F32)
            t1 = pp.tile([128, 1], F32)
            r1 = pp.tile([128, 1], F32)
            lnv = pp.tile([128, 1], F32)
            rt = pp.tile([128, 1], F32)
            t2 = pp.tile([128, 1], F32)
            r2 = pp.tile([128, 1], F32)
            alpha_bc = pp.tile([128, 1], F32)

            bounce_in = dp.tile([B, 160], BF16)
            bounce_out = dp.tile([B, 160], BF16)

            nc.vector.memset(ones[:, :], 1.0)
            # warm the ACT exp/ln table set
            nc.scalar.activation(t1[:, :], ones[:, :1], AF.Exp)
            nc.scalar.activation(t2[:, :], ones[:, :1], AF.Ln)

            nc.sync.dma_start(xt9[:, :2 * 256], xt9_d.ap()[:, :2 * 256])
            nc.sync.dma_start(wsc[:, :], wsc_d.ap())
            nc.sync.dma_start(xt9[:, 2 * 256:], xt9_d.ap()[:, 2 * 256:])
            nc.sync.dma_start(ident[:, :], ident_d.ap())

            def load_rest():
                # emitted after the r0 AllReduce kick so the bounce DMAs
                # don't queue behind these big transfers
                nc.sync.dma_start(xbp[:, :], xbp_d.ap())
                for q in range(4):
                    w = D * JP // 4
                    nc.sync.dma_start(wg[:, q * w:(q + 1) * w],
                                      wg_d.ap()[:, q * w:(q + 1) * w])

            # ---------------- helpers ----------------

            def xc_chunk(bt, dlo, dn, eng):
                """xc[:, dlo:dlo+dn digits] = c (*) x for batch-half bt."""
                xc4 = xc[:, bt * D * JP:(bt + 1) * D * JP].rearrange(
                    "m (d j p) -> m d j p", d=D, j=J, p=PL)
                cv = cb[:, bt * 1440:(bt + 1) * 1440].rearrange(
                    "m (d p) -> m d p", d=D, p=PL)
                xv = xbp[:, bt * JP:(bt + 1) * JP].rearrange(
                    "m (j p) -> m j p", j=J, p=PL)
                eng.tensor_tensor(
                    xc4[:, dlo:dlo + dn],
                    cv[:, dlo:dlo + dn, None, :].to_broadcast(
                        (128, dn, J, PL)),
                    xv[:, None, :, :].to_broadcast((128, dn, J, PL)),
                    MULT)

            def s_matmuls(r, bt, ps, dlo, dn):
                for d in range(dlo, dlo + dn):
                    for k in range(NK):
                        if r == 0:
                            lhsT = xt9[:, k * 256 + bt * 128:
                                       k * 256 + bt * 128 + 128]
                        else:
                            lhsT = xcT[:, (bt * D + d) * JP + k * 128:
                                       (bt * D + d) * JP + (k + 1) * 128]
                        nc.tensor.matmul(
                            ps[:, d * 16:(d + 1) * 16],
                            lhsT,
                            wsc[:, (d * NK + k) * 16:(d * NK + k + 1) * 16],
                            start=(k == 0), stop=(k == NK - 1))

            # xc digit groups; third group's multiply goes to Pool
            XCG = ((0, 3, nc.vector), (3, 3, nc.vector),
                   (6, 2, nc.gpsimd), (8, 2, nc.vector))

            def s_phase_front(r, bt):
                """softmax -> xc multiply -> xbar transpose for half bt.

                dma_start_transpose semantics: out[r, k, b] = in[b,
                128k + r], which lands xc^T exactly in the (d,k)-chunk
                layout the contraction matmuls want.
                """
                if r > 0:
                    softmax_c(bt)
                    for glo, gn, geng in XCG:
                        xc_chunk(bt, glo, gn, geng)
                        base = bt * D * JP
                        nc.sync.dma_start_transpose(
                            xcT[:, base + glo * JP:
                                base + (glo + gn) * JP].rearrange(
                                "m (k b) -> m k b", k=gn * NK, b=128),
                            xc[:, base + glo * JP:base + (glo + gn) * JP])

            def s_phase_mm(r, bt):
                """contraction matmuls + s drain + AllReduce kick."""
                ps = psS.tile([128, 176], F32, tag="s")
                for glo, gn, _ in XCG:
                    s_matmuls(r, bt, ps, glo, gn)
                nc.vector.tensor_copy(sbb[:, bt * 160:(bt + 1) * 160],
                                      ps[:, :160])
                bounce_bt(bt)
                return ps

            def bounce_bt(bt):
                nc.sync.dma_start(
                    bounce_in[bt * 128:(bt + 1) * 128, :],
                    sbb[:, bt * 160:(bt + 1) * 160])
                if NO_CC:
                    nc.sync.dma_start(
                        bounce_out[bt * 128:(bt + 1) * 128, :],
                        bounce_in[bt * 128:(bt + 1) * 128, :])
                else:
                    nc.gpsimd.collective_compute(
                        "AllReduce", ADD,
                        ins=[bounce_in[bt * 128:(bt + 1) * 128, :].opt()],
                        outs=[bounce_out[bt * 128:(bt + 1) * 128, :].opt()],
                        replica_groups=[list(range(CORES))],
                    )
                # land the reduced s directly in the pad-32 transpose
                # staging layout (cols d*32 + i)
                nc.sync.dma_start(
                    s_bf[:, bt * 320:(bt + 1) * 320].rearrange(
                        "m (d i) -> m d i", d=D, i=32)[:, :, :16],
                    bounce_out[bt * 128:(bt + 1) * 128, :].rearrange(
                        "m (d i) -> m d i", d=D, i=16))

            def n2_partial(bt):
                ss = s_bf[:, bt * 320:(bt + 1) * 320].rearrange(
                    "m (d i) -> m d i", d=D, i=32)[:, :, :16]
                nc.vector.tensor_tensor(
                    sq[:, bt * 160:(bt + 1) * 160].rearrange(
                        "m (d i) -> m d i", d=D, i=16), ss, ss, MULT)
                nc.vector.tensor_reduce(
                    accb[:, bt:bt + 1],
                    sq[:, None, bt * 160:(bt + 1) * 160],
                    mybir.AxisListType.X, ADD)

            def alpha_final(psn):
                # alpha = n2 / ((n2+1)(sqrt(n2)+eps)); sqrt via ln/exp so
                # ACT stays on the exp/ln table set. psn: spare cols
                # [160:161] of the last s-contract PSUM tile.
                nc.vector.tensor_tensor(acc[:, :], accb[:, 0:1], accb[:, 1:2],
                                        ADD)
                nc.tensor.matmul(psn[:, 160:161], ones[:, :], acc[:, :],
                                 start=True, stop=True)
                nc.vector.tensor_copy(n2sb[:, :], psn[:, 160:161])
                nc.vector.tensor_scalar_add(t1[:, :], n2sb[:, :], 1.0)
                nc.vector.reciprocal(r1[:, :], t1[:, :])
                nc.scalar.activation(lnv[:, :], n2sb[:, :], AF.Ln)
                nc.scalar.activation(rt[:, :], lnv[:, :], AF.Exp, scale=0.5)
                nc.vector.tensor_tensor(alpha_bc[:, :], rt[:, :], r1[:, :],
                                        MULT)

            def vtd_path(bt):
                # s^T tiles for the wv matmul lhsT; the AllReduce-return
                # DMA already landed s in s_bf's pad-32 layout.
                for rnd, (dlo, nd) in enumerate(((0, 3), (3, 3), (6, 3),
                                                 (9, 1))):
                    ptv = psV.tile([128, 128], BF16, tag="vt")
                    nc.tensor.transpose(
                        ptv[:32 * nd, :],
                        s_bf[:, bt * 320 + dlo * 32:
                             bt * 320 + (dlo + nd) * 32],
                        ident[:, :])
                    nc.vector.tensor_copy(
                        vtd[:32 * nd, rnd * 256 + bt * 128:
                            rnd * 256 + bt * 128 + 128],
                        ptv[:32 * nd, :])

            def vt_ap(d, bt):
                return vtd[32 * (d % 3):32 * (d % 3) + 16,
                           (d // 3) * 256 + bt * 128:
                           (d // 3) * 256 + bt * 128 + 128]

            # per-digit g-phase mode: ACT drain + DVE mult, ACT drain +
            # Pool mult, or DVE mult reading PSUM f32 directly (1x)
            G_MODES = (["ad"] * (D - POOL_G_D - DVE_DIRECT_D)
                       + ["dv"] * DVE_DIRECT_D + ["ap"] * POOL_G_D)

            def g_phase(bt):
                # G[b, (d,p)] = sum_j x * (W @ s^T)   (raw, no alpha)
                for d in range(D):
                    mode = G_MODES[d]
                    eng = nc.gpsimd if mode == "ap" else nc.vector
                    yb = Y[:, (d % 3) * JP:(d % 3) * JP + JP]
                    ywb = Yw[:, (d % 3) * JP:(d % 3) * JP + JP]
                    for h in range(3):
                        pw = psW.tile([128, 384], F32, tag="wv")
                        q = 32 * (d % 3)
                        nc.tensor.matmul(
                            pw[:, :], vt_ap(d, bt),
                            wg[q:q + 16,
                               d * JP + h * 384:d * JP + (h + 1) * 384],
                            start=True, stop=True)
                        xs = xbp[:, bt * JP + h * 384:bt * JP + (h + 1) * 384]
                        if mode == "dv":
                            nc.vector.tensor_tensor(
                                yb[:, h * 384:(h + 1) * 384], pw[:, :], xs,
                                MULT)
                        else:
                            nc.scalar.copy(ywb[:, h * 384:(h + 1) * 384],
                                           pw[:, :])
                            eng.tensor_tensor(
                                yb[:, h * 384:(h + 1) * 384],
                                ywb[:, h * 384:(h + 1) * 384], xs, MULT)
                    eng.tensor_tensor(yb[:, 0:576], yb[:, 0:576],
                                      yb[:, 576:1152], ADD)
                    eng.tensor_tensor(yb[:, 0:288], yb[:, 0:288],
                                      yb[:, 288:576], ADD)
                    eng.tensor_tensor(
                        G[:, bt * 1440 + d * 144:bt * 1440 + (d + 1) * 144],
                        yb[:, 0:144], yb[:, 144:288], ADD)

            def e_update(r, bt):
                gb = G[:, bt * 1440:(bt + 1) * 1440]
                eb = E[:, bt * 1440:(bt + 1) * 1440]
                if r == 0:
                    nc.scalar.activation(eb, gb, AF.Exp, scale=alpha_bc[:, :1])
                else:
                    etb = Et[:, bt * 1440:(bt + 1) * 1440]
                    nc.scalar.activation(etb, gb, AF.Exp,
                                         scale=alpha_bc[:, :1])
                    nc.vector.tensor_tensor(eb, eb, etb, MULT)

            def softmax_c(bt):
                eb = E[:, bt * 1440:(bt + 1) * 1440]
                zb = zs[:, bt * 720:(bt + 1) * 720]
                nc.vector.tensor_tensor(zb[:, 0:720], eb[:, 0:720],
                                        eb[:, 720:1440], ADD)
                nc.vector.tensor_tensor(zb[:, 0:288], zb[:, 0:288],
                                        zb[:, 288:576], ADD)
                nc.vector.tensor_tensor(zb[:, 0:144], zb[:, 0:144],
                                        zb[:, 144:288], ADD)
                nc.vector.tensor_tensor(zb[:, 0:144], zb[:, 0:144],
                                        zb[:, 576:720], ADD)
                with nc.allow_low_precision(
                        reason="softmax denominators are O(1); bf16 "
                               "reciprocal error is below the bf16 c noise"):
                    nc.vector.reciprocal(zrecb[:, bt * 144:(bt + 1) * 144],
                                         zb[:, 0:144])
                nc.vector.tensor_tensor(
                    cb[:, bt * 1440:(bt + 1) * 1440].rearrange(
                        "m (d p) -> m d p", d=D, p=PL),
                    eb.rearrange("m (d p) -> m d p", d=D, p=PL),
                    zrecb[:, None, bt * 144:(bt + 1) * 144].to_broadcast(
                        (128, D, PL)),
                    MULT)

            # ---------------- schedule ----------------
            # Emission order == per-engine execution order. Rounds are
            # software-pipelined: the next round's bt0 softmax/xc/xbar
            # front is emitted between this round's two g-phases.
            for r in range(NROUT_RUN):
                if r == 0:
                    s_phase_front(0, 0)
                    s_phase_front(0, 1)
                s_phase_mm(r, 0)
                if r == 0:
                    load_rest()
                ps = s_phase_mm(r, 1)
                last = (r == NROUT_RUN - 1)
                if last:
                    n2_partial(0)
                    n2_partial(1)
                    alpha_final(ps)
                else:
                    vtd_path(0)
                    n2_partial(0)
                    vtd_path(1)
                    g_phase(0, 0, 3)        # waits only on AllReduce(bt0)
                    n2_partial(1)
                    alpha_final(ps)         # AllReduce(bt1) lands meanwhile
                    g_phase(0, 3, D - 3)
                    e_update(r, 0)
                    s_phase_front(r + 1, 0)
                    g_phase(1)
                    e_update(r, 1)
                    s_phase_front(r + 1, 1)

            # ---- v = alpha * s ; layout already matches v_out ----
            for bt in range(2):
                nc.vector.tensor_scalar(
                    sv[:, bt * 160:(bt + 1) * 160].rearrange(
                        "m (d i) -> m d i", d=D, i=16),
                    s_bf[:, bt * 320:(bt + 1) * 320].rearrange(
                        "m (d i) -> m d i", d=D, i=32)[:, :, :16],
                    alpha_bc[:, :1], None, MULT)
                nc.sync.dma_start(v_d.ap()[bt * 128:(bt + 1) * 128, :, :],
                                  sv[:, bt * 160:(bt + 1) * 160])

    nc.compile()
    return nc


def prep_inputs(primary_caps: np.ndarray, W: np.ndarray):
    """Host-side shard + layout prep. Returns in_maps for the 8 cores."""
    x = np.asarray(primary_caps, dtype=np.float32)
    Wf = np.asarray(W, dtype=np.float32)
    bf = ml_dtypes.bfloat16
    ident = np.eye(128, dtype=np.float32).astype(bf)
    in_maps = []
    for kcore in range(CORES):
        pk = slice(kcore * PL, (kcore + 1) * PL)
        xk = x[:, pk, :]                       # [256, 144, 8]
        Wk = Wf[:, pk, :, :]                   # [10, 144, 16, 8]

        # x_bp [128, 2*1152]: row b%128, col (bt*1152 + j*144 + p)
        xbp = xk.transpose(0, 2, 1).reshape(B, JP)
        xbp = xbp.reshape(2, 128, JP).transpose(1, 0, 2).reshape(128, 2 * JP)

        # x_t9 [128, 9*256]: 0.1*x^T: row (j*144+p)%128, col k*256 + b
        xt9 = 0.1 * xk.transpose(2, 1, 0).reshape(JP, B)
        xt9 = xt9.reshape(NK, 128, B).transpose(1, 0, 2).reshape(128, NK * B)

        # w_sc [128, 10*9*16]: row (j*144+p)%128, col (d*9+k)*16 + i
        wsc = Wk.transpose(0, 3, 1, 2).reshape(D, JP, I)
        wsc = wsc.reshape(D, NK, 128, I).transpose(2, 0, 1, 3)
        wsc = wsc.reshape(128, D * NK * I)

        # w_g [128, D*J*PL]: rows 32q+i (4 replicas), col d*1152 + j*144 + p
        wg1 = Wk.transpose(2, 0, 3, 1).reshape(I, D * JP)
        wg = np.zeros((128, D * JP), dtype=np.float32)
        for q in range(3):
            wg[32 * q:32 * q + I] = wg1

        in_maps.append({
            "x_bp": xbp.astype(bf),
            "x_t9": xt9.astype(bf),
            "w_sc": wsc.astype(bf),
            "w_g": wg.astype(bf),
            "ident": ident,
        })
    return in_maps


_NC_CACHE = None


def get_program():
    global _NC_CACHE
    if _NC_CACHE is None:
        _NC_CACHE = build_program()
    return _NC_CACHE


def kernel(primary_caps: np.ndarray, W: np.ndarray) -> np.ndarray:
    nc = get_program()
    in_maps = prep_inputs(primary_caps, W)
    res = run_bass_kernel_spmd(nc, in_maps, core_ids=list(range(CORES)))
    return np.asarray(res.results[0]["v_out"], dtype=np.float32)


if __name__ == "__main__":
    rng = np.random.default_rng(0)
    x = rng.standard_normal((B, P, J), dtype=np.float32)
    W = rng.standard_normal((D, P, I, J), dtype=np.float32).astype(np.float32)
    out = kernel(x, W)
    print("out", out.shape, out.dtype, float(np.abs(out).mean()))
